# revision 25
# baseline (speedup 1.0000x reference)
"""Multi-head attention block kernel for Trainium2, sharded over 8 NeuronCores.

Sharding: batch (4) x head-group (2 groups of 8 heads) -> 8 cores.
Each core computes, for one batch b and one half of the heads:
  qh/kh/vh projections (columns of w_q/w_k/w_v for its heads),
  causal attention for its 8 heads, and a partial output projection
  (rows of w_o^T for its heads).  Host sums the two partial outputs per
  batch and transposes back.

On-chip layout is feature-major ("transposed"): activations live as
[feature, seq] so every matmul contraction dim is on partitions and no
on-chip transposes are needed.  Matmuls run in bf16; accumulation is
fp32 in PSUM.  Softmax denominators come for free from an extra ones
column appended to each V tile (row 64 of the attn@V accumulator).

Performance structure (measured on HW):
- Heads are processed in even/odd pairs living in PE-array row halves
  0:64 / 64:128: the two score matmuls carry tile_position (0,0)/(64,0)
  and execute CONCURRENTLY (PE row tiling, ~2x measured).
- exp for both heads of a pair is a single [128, 2*512] activation over
  two adjacent PSUM banks (ACT per-instruction overhead ~426ns).
- Diagonal (causally half-masked) tiles run exp and attn@V only on the
  valid q range, with one fixed 128x128 triangle mask mul.
- PSUM tags are partitioned so phase 1 of iteration i+1 (PE-heavy,
  ACT-idle) overlaps phase 2 of iteration i (ACT-bound): phase 1 owns
  a private "pp" pair-tile, phase 2 owns "scp"(x2)+"xt0/1", phase 3
  reuses "scp" (it always follows phase 2).  vh is double-buffered
  (A/B bodies) so next-iteration v-projection doesn't WAR-stall;
  qh/kh rely on subtile deps (per-ft regions release early).
"""

import sys

sys.path.insert(0, "/opt/trn_rl_repo")

import numpy as np
import ml_dtypes

import concourse.bacc as bacc
import concourse.mybir as mybir
import concourse.tile as tile
from concourse import bass_utils

B = 4
S = 2048
E = 1024
HEADS = 16
D = 64
H = 8            # heads per core
F = H * D        # 512 local head features
P = 128
ET = E // P      # 8 e-tiles
FT = F // P      # 4 f-tiles
ST = S // P      # 16 s-tiles
QC = 512         # q-chunk width
NQC = S // QC    # 4 q-chunks
KT_PER_QC = QC // P  # 4 k-tiles per q-chunk

BF16 = mybir.dt.bfloat16
F32 = mybir.dt.float32
FP8 = mybir.dt.float8e4
NPBF16 = ml_dtypes.bfloat16
NPFP8 = mybir.dt.np(FP8)
EXP = mybir.ActivationFunctionType.Exp
DR = mybir.MatmulPerfMode.DoubleRow

# fp8 is used only where quantization error stays linear (no exp
# amplification): the V projection and the output projection.  q/k stay
# bf16 (score errors get exp-amplified; measured 5e-2 rel err all-fp8).
# w_v and w_o are prescaled by WS=32 on the host (entries ~N(0, 1/1024));
# vh comes out 32x big, the ones-column denominator stays unscaled, so
# xts carries 32x and the p3 PSUM carries 32*32: the final out-copy
# multiplies by 1/(sV*sO).
FP8_V = False
FP8_O = False
WS = 32.0


def build_nc(causal: bool, niter: int | None = None, phases=(1, 2, 3),
             no_norm=False, no_exp=False, xtlag=4, fgroup=2, at_bufs=8,
             diag_narrow=True, pool_copy=False, unroll2=True, fp8_v=FP8_V,
             fp8_o=FP8_O, sc_narrow=False, tri_pool=False):
    """Build the per-core Bass program.  If niter is given, wrap the body in a
    For_i timing loop (used by test.py to measure HW time)."""
    nc = bacc.Bacc("TRN2", target_bir_lowering=False, debug=False,
                   enable_asserts=True, num_devices=8)

    VDT = FP8 if fp8_v else BF16
    ODT = FP8 if fp8_o else BF16
    qT = nc.dram_tensor("qT", [E, S], BF16, kind="ExternalInput").ap()
    kT = nc.dram_tensor("kT", [E, S], BF16, kind="ExternalInput").ap()
    vT = nc.dram_tensor("vT", [E, S], VDT, kind="ExternalInput").ap()
    wqT = nc.dram_tensor("wqT", [E, F], BF16, kind="ExternalInput").ap()
    wkT = nc.dram_tensor("wkT", [E, F], BF16, kind="ExternalInput").ap()
    wvT = nc.dram_tensor("wvT", [E, F], VDT, kind="ExternalInput").ap()
    woT = nc.dram_tensor("woT", [F, E], ODT, kind="ExternalInput").ap()
    stair = nc.dram_tensor("stair", [P, 2 * QC], BF16, kind="ExternalInput").ap()
    if not causal:
        maskT = nc.dram_tensor("maskT", [S, S], BF16, kind="ExternalInput").ap()
    outT = nc.dram_tensor("outT", [E, S], F32, kind="ExternalOutput").ap()

    qT3 = qT.rearrange("(o p) s -> p o s", p=P)
    kT3 = kT.rearrange("(o p) s -> p o s", p=P)
    vT3 = vT.rearrange("(o p) s -> p o s", p=P)
    if not causal:
        maskT3 = maskT.rearrange("(o p) s -> p o s", p=P)

    with tile.TileContext(nc) as tc:
        import contextlib
        with contextlib.ExitStack() as ctx:
            persist = ctx.enter_context(tc.tile_pool(name="persist", bufs=1))
            streams = ctx.enter_context(tc.tile_pool(name="streams", bufs=5))
            attnp = ctx.enter_context(tc.tile_pool(name="attnp", bufs=at_bufs))
            smalls = ctx.enter_context(tc.tile_pool(name="smalls", bufs=3))
            # PSUM (8 banks): scp 2x2 (p2 scores + p3) | xt0/xt1 (p2 accum)
            #                 | pp 1x2 (p1 private, enables cross-iter overlap)
            ps_sc = ctx.enter_context(tc.tile_pool(name="ps_sc", bufs=2, space="PSUM"))
            ps_xt = ctx.enter_context(tc.tile_pool(name="ps_xt", bufs=1, space="PSUM"))
            ps_pp = ctx.enter_context(tc.tile_pool(name="ps_pp", bufs=2, space="PSUM"))

            # Weights + constants: loaded once, outside the timing loop.
            wq_sb = persist.tile([P, ET, F], BF16, tag="wq")
            wk_sb = persist.tile([P, ET, F], BF16, tag="wk")
            wv_sb = persist.tile([P, ET, F], VDT, tag="wv")
            wo_sb = persist.tile([P, FT, E], ODT, tag="wo")
            stair_sb = persist.tile([P, 2 * QC], BF16, tag="stair")
            nc.sync.dma_start(wq_sb[:], wqT.rearrange("(o p) f -> p o f", p=P))
            nc.sync.dma_start(wk_sb[:], wkT.rearrange("(o p) f -> p o f", p=P))
            nc.sync.dma_start(wv_sb[:], wvT.rearrange("(o p) f -> p o f", p=P))
            nc.sync.dma_start(wo_sb[:], woT.rearrange("(o p) e -> p o e", p=P))
            nc.sync.dma_start(stair_sb[:], stair[:])
            # fixed 128x128 lower triangle: tri[kl, x] = (x >= kl)
            tri_sb = stair_sb[:, QC:QC + P]

            # Persistent activations (bf16): projections and attention outputs.
            qh_sb = persist.tile([P, FT, S], BF16, tag="qh")    # [f, ft, s]
            kh_sb = persist.tile([P, FT, S], BF16, tag="kh")
            vh_a = persist.tile([P, ST, H, D + 1], BF16, tag="vha")  # ones col at d=64
            if unroll2:
                vh_b = persist.tile([P, ST, H, D + 1], BF16, tag="vhb", name="vh_b")
            else:
                vh_b = vh_a
            xts_sb = persist.tile([P, FT, S], ODT, tag="xts")

            def pair_copy(dst2, src2):
                # src2/dst2: [128, 2, 512]-shaped pair (GPSIMD cannot read PSUM)
                if pool_copy == "act":
                    nc.vector.tensor_copy(dst2[:, 0], src2[:, 0])
                    nc.scalar.copy(dst2[:, 1], src2[:, 1])
                else:
                    nc.vector.tensor_copy(dst2[:], src2[:])

            exp_scale = 0.125
            out_scale = 1.0 / ((WS if fp8_v else 1.0) * (WS if fp8_o else 1.0))

            def body(vh_sb):
                run1 = 1 in phases
                run2 = 2 in phases
                run3 = 3 in phases
                if not run1:
                    nc.vector.memset(qh_sb[:, :, 0:1], 0.5)
                    nc.vector.memset(kh_sb[:, :, 0:1], 0.5)
                    nc.vector.memset(vh_sb[:, :, :, 0:1], 0.5)
                if not run2 and run3:
                    nc.vector.memset(xts_sb[:, :, 0:1], 0.5)

                # ---- Phase 1b: v projection -> vh (seq-major) + ones column ----
                # v first: its WAR partner (attn@V reads of the other vh
                # buffer) resolved a full iteration ago, so it overlaps the
                # previous body's ACT-bound phase 2 immediately.
                nc.vector.memset(vh_sb[:, :, :, D:D + 1], 1.0)
                for sc in range(NQC) if run1 else ():
                    xc = streams.tile([P, ET, QC], VDT, tag="xc")
                    nc.sync.dma_start(xc[:], vT3[:, :, sc * QC:(sc + 1) * QC])
                    for si in range(KT_PER_QC):
                        pp = ps_pp.tile([P, QC], F32, tag="pp", name="ppv")
                        if fp8_v:
                            for e2 in range(ET // 2):
                                nc.tensor.matmul(
                                    pp[:],
                                    xc[:, 2 * e2:2 * e2 + 2, si * P:(si + 1) * P],
                                    wv_sb[:, 2 * e2:2 * e2 + 2, :],
                                    start=(e2 == 0), stop=(e2 == ET // 2 - 1),
                                    perf_mode=DR)
                        else:
                            for et in range(ET):
                                nc.tensor.matmul(
                                    pp[:],
                                    xc[:, et, si * P:(si + 1) * P],
                                    wv_sb[:, et, :],
                                    start=(et == 0), stop=(et == ET - 1))
                        st = sc * KT_PER_QC + si
                        nc.vector.tensor_copy(
                            vh_sb[:, st, :, 0:D],
                            pp[:].rearrange("p (h d) -> p h d", h=H))

                # ---- Phase 1a: k/q projections -> kh/qh (feature-major) ----
                for src3, w_sb, dst in ((kT3, wk_sb, kh_sb), (qT3, wq_sb, qh_sb)) if run1 else ():
                    xcs = []
                    for sc in range(NQC):
                        xc = streams.tile([P, ET, QC], BF16, tag="xc")
                        nc.sync.dma_start(xc[:], src3[:, :, sc * QC:(sc + 1) * QC])
                        xcs.append(xc)
                    for ft in range(FT):
                        for sc in range(NQC):
                            pp = ps_pp.tile([P, QC], F32, tag="pp", name="pp1")
                            for et in range(ET):
                                nc.tensor.matmul(
                                    pp[:],
                                    w_sb[:, et, ft * P:(ft + 1) * P],
                                    xcs[sc][:, et, :],
                                    start=(et == 0), stop=(et == ET - 1))
                            nc.vector.tensor_copy(
                                dst[:, ft, sc * QC:(sc + 1) * QC], pp[:])

                # ---- Phase 2: attention ----
                def normalize(xt_psum, h, qc):
                    ft, fo = h // 2, (h % 2) * D
                    if no_norm:
                        nc.vector.tensor_copy(
                            xts_sb[fo:fo + D, ft, qc * QC:(qc + 1) * QC],
                            xt_psum[0:D, :])
                    else:
                        recip = smalls.tile([1, QC], F32, tag="recip")
                        nc.vector.reciprocal(recip[:], xt_psum[D:D + 1, :])
                        rb = smalls.tile([D, QC], F32, tag="rb")
                        nc.gpsimd.partition_broadcast(rb[:], recip[0:1, :])
                        nc.vector.tensor_mul(
                            xts_sb[fo:fo + D, ft, qc * QC:(qc + 1) * QC],
                            xt_psum[0:D, :], rb[:])

                if run2 and causal:
                    # Head-pair processing, one qc at a time (kt-inner).
                    for hp in range(4):
                        for qc in range(NQC):
                            ktm = (qc + 1) * KT_PER_QC
                            xt_ps = [ps_xt.tile([D + 1, QC], F32, tag=f"xt{par}",
                                                name=f"xt{par}")
                                     for par in (0, 1)]
                            pend = []  # [(kt, at, off)]

                            def flush(n):
                                # drain n generations, par-major for LDW reuse
                                gens = [pend.pop(0) for _ in range(n)]
                                for par in (0, 1):
                                    for kt2, at, off in gens:
                                        nc.tensor.matmul(
                                            xt_ps[par][:, off:],
                                            vh_sb[:, kt2, 2 * hp + par, :],
                                            at[:, par, off:],
                                            start=(kt2 == 0),
                                            stop=(kt2 == ktm - 1))

                            for kt in range(ktm):
                                diag = (kt // KT_PER_QC == qc)
                                soff = ((kt % KT_PER_QC) * P
                                        if (diag and diag_narrow and sc_narrow)
                                        else 0)
                                scp = ps_sc.tile([P, 2, QC], F32, tag="scp",
                                                 name="scp")
                                for par in (0, 1):
                                    nc.tensor.matmul(
                                        scp[:, par, soff:],
                                        kh_sb[par * D:(par + 1) * D, hp,
                                              kt * P:(kt + 1) * P],
                                        qh_sb[par * D:(par + 1) * D, hp,
                                              qc * QC + soff:(qc + 1) * QC],
                                        start=True, stop=True)
                                at = attnp.tile([P, 2, QC], BF16, tag="at",
                                                name="at")
                                off = (kt % KT_PER_QC) * P if (diag and diag_narrow) else 0
                                if no_exp:
                                    nc.vector.tensor_copy(
                                        at[:, :, off:], scp[:, :, off:])
                                else:
                                    nc.scalar.activation(
                                        at[:, :, off:], scp[:, :, off:],
                                        EXP, scale=exp_scale)
                                if diag:
                                    o2 = (kt % KT_PER_QC) * P
                                    eng = nc.gpsimd if tri_pool else nc.vector
                                    for par in (0, 1):
                                        eng.tensor_mul(
                                            at[:, par, o2:o2 + P],
                                            at[:, par, o2:o2 + P],
                                            tri_sb)
                                    if not diag_narrow and o2 > 0:
                                        nc.vector.memset(at[:, :, 0:o2], 0.0)
                                pend.append((kt, at, off))
                                if len(pend) > xtlag:
                                    flush(min(fgroup, len(pend)))
                            flush(len(pend))
                            for par in (0, 1):
                                normalize(xt_ps[par], 2 * hp + par, qc)

                elif run2:
                    # general-mask path: qc-outer, mask tiles streamed per qc.
                    for qc in range(NQC):
                        mc = streams.tile([P, ST, QC], BF16, tag="mc")
                        nc.sync.dma_start(mc[:], maskT3[:, :, qc * QC:(qc + 1) * QC])
                        ktm = ST
                        for h in range(H):
                            ft, fo = h // 2, (h % 2) * D
                            xt_psum = ps_xt.tile([D + 1, QC], F32, tag="xt0")
                            at_tiles = [None] * ktm

                            def emit_sc(kt):
                                scp = ps_sc.tile([P, 2, QC], F32, tag="scp")
                                nc.tensor.matmul(
                                    scp[:, 0, :],
                                    kh_sb[fo:fo + D, ft, kt * P:(kt + 1) * P],
                                    qh_sb[fo:fo + D, ft, qc * QC:(qc + 1) * QC],
                                    start=True, stop=True)
                                at = attnp.tile([P, 2, QC], BF16, tag="at")
                                if no_exp:
                                    nc.vector.tensor_copy(at[:, 0, :], scp[:, 0, :])
                                else:
                                    nc.scalar.activation(at[:, 0, :], scp[:, 0, :],
                                                         EXP, scale=exp_scale)
                                nc.vector.tensor_mul(at[:, 0, :], at[:, 0, :],
                                                     mc[:, kt, :])
                                at_tiles[kt] = at

                            def emit_xt(kt):
                                nc.tensor.matmul(
                                    xt_psum[:],
                                    vh_sb[:, kt, h, :],
                                    at_tiles[kt][:, 0, :],
                                    start=(kt == 0), stop=(kt == ktm - 1))

                            PIPE = 2
                            for kt in range(ktm):
                                emit_sc(kt)
                                if kt >= PIPE:
                                    emit_xt(kt - PIPE)
                            for kt in range(max(0, ktm - PIPE), ktm):
                                emit_xt(kt)
                            normalize(xt_psum, h, qc)

                # ---- Phase 3: output projection (partial over local heads) ----
                for jt in range(ET) if run3 else ():
                    scps = [ps_sc.tile([P, 2, QC], F32, tag="scp", name=f"p3{g}")
                            for g in range(2)]
                    if fp8_o:
                        for f2 in range(FT // 2):
                            for qcq in range(NQC):
                                nc.tensor.matmul(
                                    scps[qcq // 2][:, qcq % 2, :],
                                    wo_sb[:, 2 * f2:2 * f2 + 2,
                                          jt * P:(jt + 1) * P],
                                    xts_sb[:, 2 * f2:2 * f2 + 2,
                                           qcq * QC:(qcq + 1) * QC],
                                    start=(f2 == 0), stop=(f2 == FT // 2 - 1),
                                    perf_mode=DR)
                    else:
                        for ft in range(FT):
                            for qcq in range(NQC):
                                nc.tensor.matmul(
                                    scps[qcq // 2][:, qcq % 2, :],
                                    wo_sb[:, ft, jt * P:(jt + 1) * P],
                                    xts_sb[:, ft, qcq * QC:(qcq + 1) * QC],
                                    start=(ft == 0), stop=(ft == FT - 1))
                    for g in range(2):
                        ot = streams.tile([P, 2 * QC], F32, tag="ot", bufs=3)
                        if out_scale != 1.0:
                            nc.vector.tensor_scalar_mul(
                                ot[:].rearrange("p (a b) -> p a b", a=2),
                                scps[g][:], out_scale)
                        else:
                            nc.vector.tensor_copy(
                                ot[:].rearrange("p (a b) -> p a b", a=2),
                                scps[g][:])
                        nc.sync.dma_start(
                            outT[jt * P:(jt + 1) * P, g * 2 * QC:(g + 1) * 2 * QC],
                            ot[:])

            if niter is None:
                body(vh_a)
            elif unroll2:
                assert niter % 2 == 0
                with tc.For_i(0, niter // 2, 1):
                    body(vh_a)
                    body(vh_b)
            else:
                with tc.For_i(0, niter, 1):
                    body(vh_a)

    nc.compile()
    return nc


def _host_prep(q, k, v, mask, w_q, w_k, w_v, w_o):
    """Shard + transpose inputs on the host.  Returns (in_maps, causal)."""
    tril = np.tril(np.ones((S, S), dtype=mask.dtype))
    causal = all(np.array_equal(np.asarray(mask[b, 0]), tril) for b in range(B))

    stair = (np.arange(2 * QC)[None, :] >= (np.arange(P)[:, None] + QC))
    stair = stair.astype(NPBF16)

    w_q = np.asarray(w_q, dtype=np.float32)
    w_k = np.asarray(w_k, dtype=np.float32)
    w_v = np.asarray(w_v, dtype=np.float32)
    w_o = np.asarray(w_o, dtype=np.float32)
    if FP8_V:
        w_v = w_v * WS
    if FP8_O:
        w_o = w_o * WS

    in_maps = []
    for core in range(8):
        b, g = divmod(core, 2)
        rows = slice(g * F, (g + 1) * F)
        NPV = NPFP8 if FP8_V else NPBF16
        NPO = NPFP8 if FP8_O else NPBF16
        m = {
            "qT": np.ascontiguousarray(np.asarray(q[b], np.float32).T).astype(NPBF16),
            "kT": np.ascontiguousarray(np.asarray(k[b], np.float32).T).astype(NPBF16),
            "vT": np.ascontiguousarray(np.asarray(v[b], np.float32).T).astype(NPV),
            "wqT": np.ascontiguousarray(w_q[rows, :].T).astype(NPBF16),
            "wkT": np.ascontiguousarray(w_k[rows, :].T).astype(NPBF16),
            "wvT": np.ascontiguousarray(w_v[rows, :].T).astype(NPV),
            "woT": np.ascontiguousarray(w_o[:, rows].T).astype(NPO),
            "stair": stair,
        }
        if not causal:
            m["maskT"] = np.ascontiguousarray(
                np.asarray(mask[b, 0], np.float32).T).astype(NPBF16)
        in_maps.append(m)
    return in_maps, causal


_NC_CACHE: dict = {}


def kernel(q, k, v, mask, w_q, w_k, w_v, w_o):
    in_maps, causal = _host_prep(q, k, v, mask, w_q, w_k, w_v, w_o)
    nc = _NC_CACHE.get(causal)
    if nc is None:
        nc = build_nc(causal)
        _NC_CACHE[causal] = nc
    res = bass_utils.run_bass_kernel_spmd(nc, in_maps, core_ids=list(range(8)))
    out = np.empty((B, S, E), dtype=np.float32)
    for b in range(B):
        out[b] = (res.results[2 * b]["outT"] + res.results[2 * b + 1]["outT"]).T
    return out


# revision 26
# speedup vs baseline: 1.6610x; 1.6610x over previous
"""Multi-head attention block kernel for Trainium2, sharded over 8 NeuronCores.

Sharding: batch (4) x head-group (2 groups of 8 heads) -> 8 cores.
Each core computes, for one batch b and one half of the heads:
  qh/kh/vh projections (columns of w_q/w_k/w_v for its heads),
  causal attention for its 8 heads, and a partial output projection
  (rows of w_o^T for its heads).  Host sums the two partial outputs per
  batch and transposes back.

On-chip layout is feature-major ("transposed"): activations live as
[feature, seq] so every matmul contraction dim is on partitions and no
on-chip transposes are needed.  Matmuls run in bf16; accumulation is
fp32 in PSUM.  Softmax denominators come for free from an extra ones
column appended to each V tile (row 64 of the attn@V accumulator).

Performance structure (measured on HW):
- Heads are processed in even/odd pairs living in PE-array row halves
  0:64 / 64:128: the two score matmuls carry tile_position (0,0)/(64,0)
  and execute CONCURRENTLY (PE row tiling, ~2x measured).
- exp for both heads of a pair is a single [128, 2*512] activation over
  two adjacent PSUM banks (ACT per-instruction overhead ~426ns).
- Diagonal (causally half-masked) tiles run exp and attn@V only on the
  valid q range, with one fixed 128x128 triangle mask mul.
- PSUM tags are partitioned so phase 1 of iteration i+1 (PE-heavy,
  ACT-idle) overlaps phase 2 of iteration i (ACT-bound): phase 1 owns
  a private "pp" pair-tile, phase 2 owns "scp"(x2)+"xt0/1", phase 3
  reuses "scp" (it always follows phase 2).  vh is double-buffered
  (A/B bodies) so next-iteration v-projection doesn't WAR-stall;
  qh/kh rely on subtile deps (per-ft regions release early).
"""

import sys

sys.path.insert(0, "/opt/trn_rl_repo")

import numpy as np
import ml_dtypes

import concourse.bacc as bacc
import concourse.mybir as mybir
import concourse.tile as tile
from concourse import bass_utils

B = 4
S = 2048
E = 1024
HEADS = 16
D = 64
H = 8            # heads per core
F = H * D        # 512 local head features
P = 128
ET = E // P      # 8 e-tiles
FT = F // P      # 4 f-tiles
ST = S // P      # 16 s-tiles
QC = 512         # q-chunk width
NQC = S // QC    # 4 q-chunks
KT_PER_QC = QC // P  # 4 k-tiles per q-chunk

BF16 = mybir.dt.bfloat16
F32 = mybir.dt.float32
FP8 = mybir.dt.float8e4
NPBF16 = ml_dtypes.bfloat16
NPFP8 = mybir.dt.np(FP8)
EXP = mybir.ActivationFunctionType.Exp
DR = mybir.MatmulPerfMode.DoubleRow

# fp8 is used only where quantization error stays linear (no exp
# amplification): the V projection and the output projection.  q/k stay
# bf16 (score errors get exp-amplified; measured 5e-2 rel err all-fp8).
# w_v and w_o are prescaled by WS=32 on the host (entries ~N(0, 1/1024));
# vh comes out 32x big, the ones-column denominator stays unscaled, so
# xts carries 32x and the p3 PSUM carries 32*32: the final out-copy
# multiplies by 1/(sV*sO).
FP8_V = False
FP8_O = False
WS = 32.0


def build_nc(causal: bool, niter: int | None = None, phases=(1, 2, 3),
             no_norm=False, no_exp=False, xtlag=4, fgroup=2, at_bufs=8,
             diag_narrow=True, pool_copy=False, unroll2=True, fp8_v=FP8_V,
             fp8_o=FP8_O, sc_narrow=False, tri_pool=False, xt_full=False):
    """Build the per-core Bass program.  If niter is given, wrap the body in a
    For_i timing loop (used by test.py to measure HW time)."""
    nc = bacc.Bacc("TRN2", target_bir_lowering=False, debug=False,
                   enable_asserts=True, num_devices=8)

    VDT = FP8 if fp8_v else BF16
    ODT = FP8 if fp8_o else BF16
    qT = nc.dram_tensor("qT", [E, S], BF16, kind="ExternalInput").ap()
    kT = nc.dram_tensor("kT", [E, S], BF16, kind="ExternalInput").ap()
    vT = nc.dram_tensor("vT", [E, S], VDT, kind="ExternalInput").ap()
    wqT = nc.dram_tensor("wqT", [E, F], BF16, kind="ExternalInput").ap()
    wkT = nc.dram_tensor("wkT", [E, F], BF16, kind="ExternalInput").ap()
    wvT = nc.dram_tensor("wvT", [E, F], VDT, kind="ExternalInput").ap()
    woT = nc.dram_tensor("woT", [F, E], ODT, kind="ExternalInput").ap()
    stair = nc.dram_tensor("stair", [P, 2 * QC], BF16, kind="ExternalInput").ap()
    if not causal:
        maskT = nc.dram_tensor("maskT", [S, S], BF16, kind="ExternalInput").ap()
    outT = nc.dram_tensor("outT", [E, S], F32, kind="ExternalOutput").ap()

    qT3 = qT.rearrange("(o p) s -> p o s", p=P)
    kT3 = kT.rearrange("(o p) s -> p o s", p=P)
    vT3 = vT.rearrange("(o p) s -> p o s", p=P)
    if not causal:
        maskT3 = maskT.rearrange("(o p) s -> p o s", p=P)

    with tile.TileContext(nc) as tc:
        import contextlib
        with contextlib.ExitStack() as ctx:
            persist = ctx.enter_context(tc.tile_pool(name="persist", bufs=1))
            streams = ctx.enter_context(tc.tile_pool(name="streams", bufs=5))
            attnp = ctx.enter_context(tc.tile_pool(name="attnp", bufs=at_bufs))
            smalls = ctx.enter_context(tc.tile_pool(name="smalls", bufs=3))
            # PSUM (8 banks): scp 2x2 (p2 scores + p3) | xt0/xt1 (p2 accum)
            #                 | pp 1x2 (p1 private, enables cross-iter overlap)
            ps_sc = ctx.enter_context(tc.tile_pool(name="ps_sc", bufs=2, space="PSUM"))
            ps_xt = ctx.enter_context(tc.tile_pool(name="ps_xt", bufs=1, space="PSUM"))
            ps_pp = ctx.enter_context(tc.tile_pool(name="ps_pp", bufs=2, space="PSUM"))

            # Weights + constants: loaded once, outside the timing loop.
            wq_sb = persist.tile([P, ET, F], BF16, tag="wq")
            wk_sb = persist.tile([P, ET, F], BF16, tag="wk")
            wv_sb = persist.tile([P, ET, F], VDT, tag="wv")
            wo_sb = persist.tile([P, FT, E], ODT, tag="wo")
            stair_sb = persist.tile([P, 2 * QC], BF16, tag="stair")
            nc.sync.dma_start(wq_sb[:], wqT.rearrange("(o p) f -> p o f", p=P))
            nc.sync.dma_start(wk_sb[:], wkT.rearrange("(o p) f -> p o f", p=P))
            nc.sync.dma_start(wv_sb[:], wvT.rearrange("(o p) f -> p o f", p=P))
            nc.sync.dma_start(wo_sb[:], woT.rearrange("(o p) e -> p o e", p=P))
            nc.sync.dma_start(stair_sb[:], stair[:])
            # fixed 128x128 lower triangle: tri[kl, x] = (x >= kl)
            tri_sb = stair_sb[:, QC:QC + P]

            # Persistent activations (bf16): projections and attention outputs.
            qh_sb = persist.tile([P, FT, S], BF16, tag="qh")    # [f, ft, s]
            kh_sb = persist.tile([P, FT, S], BF16, tag="kh")
            vh_a = persist.tile([P, ST, H, D + 1], BF16, tag="vha")  # ones col at d=64
            if unroll2:
                vh_b = persist.tile([P, ST, H, D + 1], BF16, tag="vhb", name="vh_b")
            else:
                vh_b = vh_a
            xts_sb = persist.tile([P, FT, S], ODT, tag="xts")

            def pair_copy(dst2, src2):
                # src2/dst2: [128, 2, 512]-shaped pair (GPSIMD cannot read PSUM)
                if pool_copy == "act":
                    nc.vector.tensor_copy(dst2[:, 0], src2[:, 0])
                    nc.scalar.copy(dst2[:, 1], src2[:, 1])
                else:
                    nc.vector.tensor_copy(dst2[:], src2[:])

            exp_scale = 0.125
            out_scale = 1.0 / ((WS if fp8_v else 1.0) * (WS if fp8_o else 1.0))

            def body(vh_sb):
                run1 = 1 in phases
                run2 = 2 in phases
                run3 = 3 in phases
                if not run1:
                    nc.vector.memset(qh_sb[:, :, 0:1], 0.5)
                    nc.vector.memset(kh_sb[:, :, 0:1], 0.5)
                    nc.vector.memset(vh_sb[:, :, :, 0:1], 0.5)
                if not run2 and run3:
                    nc.vector.memset(xts_sb[:, :, 0:1], 0.5)

                # ---- Phase 1b: v projection -> vh (seq-major) + ones column ----
                # v first: its WAR partner (attn@V reads of the other vh
                # buffer) resolved a full iteration ago, so it overlaps the
                # previous body's ACT-bound phase 2 immediately.
                nc.vector.memset(vh_sb[:, :, :, D:D + 1], 1.0)
                for sc in range(NQC) if run1 else ():
                    xc = streams.tile([P, ET, QC], VDT, tag="xc")
                    nc.sync.dma_start(xc[:], vT3[:, :, sc * QC:(sc + 1) * QC])
                    for si in range(KT_PER_QC):
                        pp = ps_pp.tile([P, QC], F32, tag="pp", name="ppv")
                        if fp8_v:
                            for e2 in range(ET // 2):
                                nc.tensor.matmul(
                                    pp[:],
                                    xc[:, 2 * e2:2 * e2 + 2, si * P:(si + 1) * P],
                                    wv_sb[:, 2 * e2:2 * e2 + 2, :],
                                    start=(e2 == 0), stop=(e2 == ET // 2 - 1),
                                    perf_mode=DR)
                        else:
                            for et in range(ET):
                                nc.tensor.matmul(
                                    pp[:],
                                    xc[:, et, si * P:(si + 1) * P],
                                    wv_sb[:, et, :],
                                    start=(et == 0), stop=(et == ET - 1))
                        st = sc * KT_PER_QC + si
                        nc.vector.tensor_copy(
                            vh_sb[:, st, :, 0:D],
                            pp[:].rearrange("p (h d) -> p h d", h=H))

                # ---- Phase 1a: k/q projections -> kh/qh (feature-major) ----
                for src3, w_sb, dst in ((kT3, wk_sb, kh_sb), (qT3, wq_sb, qh_sb)) if run1 else ():
                    xcs = []
                    for sc in range(NQC):
                        xc = streams.tile([P, ET, QC], BF16, tag="xc")
                        nc.sync.dma_start(xc[:], src3[:, :, sc * QC:(sc + 1) * QC])
                        xcs.append(xc)
                    for ft in range(FT):
                        for sc in range(NQC):
                            pp = ps_pp.tile([P, QC], F32, tag="pp", name="pp1")
                            for et in range(ET):
                                nc.tensor.matmul(
                                    pp[:],
                                    w_sb[:, et, ft * P:(ft + 1) * P],
                                    xcs[sc][:, et, :],
                                    start=(et == 0), stop=(et == ET - 1))
                            nc.vector.tensor_copy(
                                dst[:, ft, sc * QC:(sc + 1) * QC], pp[:])

                # ---- Phase 2: attention ----
                def normalize(xt_psum, h, qc):
                    ft, fo = h // 2, (h % 2) * D
                    if no_norm:
                        nc.vector.tensor_copy(
                            xts_sb[fo:fo + D, ft, qc * QC:(qc + 1) * QC],
                            xt_psum[0:D, :])
                    else:
                        recip = smalls.tile([1, QC], F32, tag="recip")
                        nc.vector.reciprocal(recip[:], xt_psum[D:D + 1, :])
                        rb = smalls.tile([D, QC], F32, tag="rb")
                        nc.gpsimd.partition_broadcast(rb[:], recip[0:1, :])
                        nc.vector.tensor_mul(
                            xts_sb[fo:fo + D, ft, qc * QC:(qc + 1) * QC],
                            xt_psum[0:D, :], rb[:])

                if run2 and causal:
                    # Head-pair processing, one qc at a time (kt-inner).
                    for hp in range(4):
                        for qc in range(NQC):
                            ktm = (qc + 1) * KT_PER_QC
                            xt_ps = [ps_xt.tile([D + 1, QC], F32, tag=f"xt{par}",
                                                name=f"xt{par}")
                                     for par in (0, 1)]
                            pend = []  # [(kt, at, off)]

                            def flush(n):
                                # drain n generations, par-major for LDW reuse
                                gens = [pend.pop(0) for _ in range(n)]
                                for par in (0, 1):
                                    for kt2, at, off in gens:
                                        nc.tensor.matmul(
                                            xt_ps[par][:, off:],
                                            vh_sb[:, kt2, 2 * hp + par, :],
                                            at[:, par, off:],
                                            start=(kt2 == 0),
                                            stop=(kt2 == ktm - 1))

                            for kt in range(ktm):
                                diag = (kt // KT_PER_QC == qc)
                                soff = ((kt % KT_PER_QC) * P
                                        if (diag and diag_narrow and sc_narrow)
                                        else 0)
                                scp = ps_sc.tile([P, 2, QC], F32, tag="scp",
                                                 name="scp")
                                for par in (0, 1):
                                    nc.tensor.matmul(
                                        scp[:, par, soff:],
                                        kh_sb[par * D:(par + 1) * D, hp,
                                              kt * P:(kt + 1) * P],
                                        qh_sb[par * D:(par + 1) * D, hp,
                                              qc * QC + soff:(qc + 1) * QC],
                                        start=True, stop=True)
                                at = attnp.tile([P, 2, QC], BF16, tag="at",
                                                name="at")
                                off = (kt % KT_PER_QC) * P if (diag and diag_narrow) else 0
                                if no_exp:
                                    nc.vector.tensor_copy(
                                        at[:, :, off:], scp[:, :, off:])
                                else:
                                    nc.scalar.activation(
                                        at[:, :, off:], scp[:, :, off:],
                                        EXP, scale=exp_scale)
                                if diag:
                                    o2 = (kt % KT_PER_QC) * P
                                    eng = nc.gpsimd if tri_pool else nc.vector
                                    for par in (0, 1):
                                        eng.tensor_mul(
                                            at[:, par, o2:o2 + P],
                                            at[:, par, o2:o2 + P],
                                            tri_sb)
                                    if (xt_full or not diag_narrow) and o2 > 0:
                                        nc.vector.memset(at[:, :, 0:o2], 0.0)
                                pend.append((kt, at, 0 if xt_full else off))
                                if len(pend) > xtlag:
                                    flush(min(fgroup, len(pend)))
                            flush(len(pend))
                            for par in (0, 1):
                                normalize(xt_ps[par], 2 * hp + par, qc)

                elif run2:
                    # general-mask path: qc-outer, mask tiles streamed per qc.
                    for qc in range(NQC):
                        mc = streams.tile([P, ST, QC], BF16, tag="mc")
                        nc.sync.dma_start(mc[:], maskT3[:, :, qc * QC:(qc + 1) * QC])
                        ktm = ST
                        for h in range(H):
                            ft, fo = h // 2, (h % 2) * D
                            xt_psum = ps_xt.tile([D + 1, QC], F32, tag="xt0")
                            at_tiles = [None] * ktm

                            def emit_sc(kt):
                                scp = ps_sc.tile([P, 2, QC], F32, tag="scp")
                                nc.tensor.matmul(
                                    scp[:, 0, :],
                                    kh_sb[fo:fo + D, ft, kt * P:(kt + 1) * P],
                                    qh_sb[fo:fo + D, ft, qc * QC:(qc + 1) * QC],
                                    start=True, stop=True)
                                at = attnp.tile([P, 2, QC], BF16, tag="at")
                                if no_exp:
                                    nc.vector.tensor_copy(at[:, 0, :], scp[:, 0, :])
                                else:
                                    nc.scalar.activation(at[:, 0, :], scp[:, 0, :],
                                                         EXP, scale=exp_scale)
                                nc.vector.tensor_mul(at[:, 0, :], at[:, 0, :],
                                                     mc[:, kt, :])
                                at_tiles[kt] = at

                            def emit_xt(kt):
                                nc.tensor.matmul(
                                    xt_psum[:],
                                    vh_sb[:, kt, h, :],
                                    at_tiles[kt][:, 0, :],
                                    start=(kt == 0), stop=(kt == ktm - 1))

                            PIPE = 2
                            for kt in range(ktm):
                                emit_sc(kt)
                                if kt >= PIPE:
                                    emit_xt(kt - PIPE)
                            for kt in range(max(0, ktm - PIPE), ktm):
                                emit_xt(kt)
                            normalize(xt_psum, h, qc)

                # ---- Phase 3: output projection (partial over local heads) ----
                for jt in range(ET) if run3 else ():
                    scps = [ps_sc.tile([P, 2, QC], F32, tag="scp", name=f"p3{g}")
                            for g in range(2)]
                    if fp8_o:
                        for f2 in range(FT // 2):
                            for qcq in range(NQC):
                                nc.tensor.matmul(
                                    scps[qcq // 2][:, qcq % 2, :],
                                    wo_sb[:, 2 * f2:2 * f2 + 2,
                                          jt * P:(jt + 1) * P],
                                    xts_sb[:, 2 * f2:2 * f2 + 2,
                                           qcq * QC:(qcq + 1) * QC],
                                    start=(f2 == 0), stop=(f2 == FT // 2 - 1),
                                    perf_mode=DR)
                    else:
                        for ft in range(FT):
                            for qcq in range(NQC):
                                nc.tensor.matmul(
                                    scps[qcq // 2][:, qcq % 2, :],
                                    wo_sb[:, ft, jt * P:(jt + 1) * P],
                                    xts_sb[:, ft, qcq * QC:(qcq + 1) * QC],
                                    start=(ft == 0), stop=(ft == FT - 1))
                    for g in range(2):
                        ot = streams.tile([P, 2 * QC], F32, tag="ot", bufs=3)
                        if out_scale != 1.0:
                            nc.vector.tensor_scalar_mul(
                                ot[:].rearrange("p (a b) -> p a b", a=2),
                                scps[g][:], out_scale)
                        else:
                            nc.vector.tensor_copy(
                                ot[:].rearrange("p (a b) -> p a b", a=2),
                                scps[g][:])
                        nc.sync.dma_start(
                            outT[jt * P:(jt + 1) * P, g * 2 * QC:(g + 1) * 2 * QC],
                            ot[:])

            if niter is None:
                body(vh_a)
            elif unroll2:
                assert niter % 2 == 0
                with tc.For_i(0, niter // 2, 1):
                    body(vh_a)
                    body(vh_b)
            else:
                with tc.For_i(0, niter, 1):
                    body(vh_a)

    nc.compile()
    return nc


def _host_prep(q, k, v, mask, w_q, w_k, w_v, w_o):
    """Shard + transpose inputs on the host.  Returns (in_maps, causal)."""
    tril = np.tril(np.ones((S, S), dtype=mask.dtype))
    causal = all(np.array_equal(np.asarray(mask[b, 0]), tril) for b in range(B))

    stair = (np.arange(2 * QC)[None, :] >= (np.arange(P)[:, None] + QC))
    stair = stair.astype(NPBF16)

    w_q = np.asarray(w_q, dtype=np.float32)
    w_k = np.asarray(w_k, dtype=np.float32)
    w_v = np.asarray(w_v, dtype=np.float32)
    w_o = np.asarray(w_o, dtype=np.float32)
    if FP8_V:
        w_v = w_v * WS
    if FP8_O:
        w_o = w_o * WS

    in_maps = []
    for core in range(8):
        b, g = divmod(core, 2)
        rows = slice(g * F, (g + 1) * F)
        NPV = NPFP8 if FP8_V else NPBF16
        NPO = NPFP8 if FP8_O else NPBF16
        m = {
            "qT": np.ascontiguousarray(np.asarray(q[b], np.float32).T).astype(NPBF16),
            "kT": np.ascontiguousarray(np.asarray(k[b], np.float32).T).astype(NPBF16),
            "vT": np.ascontiguousarray(np.asarray(v[b], np.float32).T).astype(NPV),
            "wqT": np.ascontiguousarray(w_q[rows, :].T).astype(NPBF16),
            "wkT": np.ascontiguousarray(w_k[rows, :].T).astype(NPBF16),
            "wvT": np.ascontiguousarray(w_v[rows, :].T).astype(NPV),
            "woT": np.ascontiguousarray(w_o[:, rows].T).astype(NPO),
            "stair": stair,
        }
        if not causal:
            m["maskT"] = np.ascontiguousarray(
                np.asarray(mask[b, 0], np.float32).T).astype(NPBF16)
        in_maps.append(m)
    return in_maps, causal


_NC_CACHE: dict = {}


def kernel(q, k, v, mask, w_q, w_k, w_v, w_o):
    in_maps, causal = _host_prep(q, k, v, mask, w_q, w_k, w_v, w_o)
    nc = _NC_CACHE.get(causal)
    if nc is None:
        nc = build_nc(causal)
        _NC_CACHE[causal] = nc
    res = bass_utils.run_bass_kernel_spmd(nc, in_maps, core_ids=list(range(8)))
    out = np.empty((B, S, E), dtype=np.float32)
    for b in range(B):
        out[b] = (res.results[2 * b]["outT"] + res.results[2 * b + 1]["outT"]).T
    return out
